# revision 13
# baseline (speedup 1.0000x reference)
"""Multi-head attention block (QKV proj + RMSNorm + RoPE + SDPA + out proj)
sharded across 8 Trainium2 NeuronCores.

Sharding: data-parallel over batch (B=2 -> 2 groups of 4 cores), tensor-parallel
over heads (16 heads -> 4 heads/core).  Each core computes a partial output
projection for its 4 heads; the host sums the 4 partials per batch and adds
bproj.

Device layout strategy (per core, batch b, heads hg*4..hg*4+3):
  - qkv computed transposed (d-on-partitions) so scores/attn matmuls need no
    on-chip transposes:  qT,kT [4*64, 2048],  v natural [2048, 4*64 | ones].
  - RMSNorm: sum-of-squares via mask-matmul (partition reduction on PE, sums
    at psum partitions {0,32}), then a single ACT Rsqrt per [33,512] block.
    The rsqrt rows are broadcast to 64 partitions by a stride-0 SBUF DMA and
    applied with one DVE multiply BEFORE RoPE (the norm scale is constant
    within a head at fixed position, so it commutes with the rotation).
  - RoPE half-rotation via a signed permutation matmul on PE.
  - Attention processes ONE head at a time with k-chunks PAIRED across PE
    row-groups (kdup holds even chunks at partitions 0-63, odd at 64-127;
    qdup duplicates q to both halves).  Each pair: 4 concurrent score
    matmuls -> 2 [128,1024] exps (ACT) -> 4 attn@v matmuls accumulating in
    one [65,1024] psum tile.  PSUM: 3 score slots (6 banks) + 1 accumulator
    (2 banks) = 8 banks, letting scores for pair j+1 overlap exp of pair j
    so the phase runs at the ACT exp roofline.
  - softmax denominator comes free from an appended ones-column on V
    (row 64 of the [65, q] attn@v accumulator); 1/denom via the fast
    custom-DVE reciprocal, broadcast back with a stride-0 SBUF DMA.
"""

import numpy as np
import ml_dtypes

B, S, D, H = 2, 2048, 1024, 16
HD = D // H
EPS = 1e-6
N_CORES = 8
HPC = H // 4  # heads per core = 4
CW = HPC * HD  # per-core head-col width = 256

BF16 = ml_dtypes.bfloat16

LAST_RESULTS = None  # stash of BassKernelResults for test harness introspection


def _build_bass(debug=False, stage="F", reps=1):
    """stage: progressively include phases: 'B' < 'D' < 'E' < 'F'.
    reps>1 repeats the compute phases (benchmarking: marginal time per rep
    = steady-state compute time, input DMAs amortized)."""
    import concourse.mybir as mybir
    import concourse.tile as tile
    from concourse import bacc

    fp32 = mybir.dt.float32
    bf16 = mybir.dt.bfloat16
    AF = mybir.ActivationFunctionType

    nc = bacc.Bacc()

    # ---- DRAM I/O ----
    xTa = nc.dram_tensor("xTa", [1152, S], bf16, kind="ExternalInput")
    wqk = nc.dram_tensor("wqk", [1152, 2 * CW], bf16, kind="ExternalInput")
    wv = nc.dram_tensor("wv", [1152, CW], bf16, kind="ExternalInput")
    wpr = nc.dram_tensor("wpr", [CW, D], bf16, kind="ExternalInput")
    cosT2 = nc.dram_tensor("cosT2", [128, S], bf16, kind="ExternalInput")
    sinT2 = nc.dram_tensor("sinT2", [128, S], bf16, kind="ExternalInput")
    mask33 = nc.dram_tensor("mask33", [128, 33], bf16, kind="ExternalInput")
    ones4 = nc.dram_tensor("ones4", [128, 64], fp32, kind="ExternalInput")
    perm = nc.dram_tensor("perm", [128, 128], bf16, kind="ExternalInput")
    out = nc.dram_tensor("out", [S, D], fp32, kind="ExternalOutput")
    if debug:
        dbg_qkT = nc.dram_tensor("dbg_qkT", [128, 4, S], bf16, kind="ExternalOutput")
        dbg_rsq = nc.dram_tensor("dbg_rsq", [64, 4, S], fp32, kind="ExternalOutput")
        dbg_qdup = nc.dram_tensor("dbg_qdup", [128, 4, S], bf16, kind="ExternalOutput")
        dbg_kdup = nc.dram_tensor("dbg_kdup", [128, 4, 8, 128], bf16, kind="ExternalOutput")
        dbg_v = nc.dram_tensor("dbg_v", [128, 16, 264], bf16, kind="ExternalOutput")
        dbg_oT = nc.dram_tensor("dbg_oT", [128, 2, S], bf16, kind="ExternalOutput")
        dbg_rd = nc.dram_tensor("dbg_rd", [32, S], fp32, kind="ExternalOutput")

    NSEG = S // 512  # 4

    with tile.TileContext(nc) as tc:
        with tc.tile_pool(name="persist", bufs=1) as pp:
            # persistent SBUF tensors
            xTa_sb = pp.tile([128, 9, S], bf16, name="xTa_sb")
            wqk_sb = pp.tile([128, 9, 2 * CW], bf16, name="wqk_sb")
            wv_sb = pp.tile([128, 9, CW], bf16, name="wv_sb")
            wpr_sb = pp.tile([128, 2, D], bf16, name="wpr_sb")
            cos_sb = pp.tile([128, S], bf16, name="cos_sb")
            sin_sb = pp.tile([128, S], bf16, name="sin_sb")
            mask_sb = pp.tile([128, 33], bf16, name="mask_sb")
            ones4_sb = pp.tile([128, 64], fp32, name="ones4_sb")
            perm_sb = pp.tile([128, 128], bf16, name="perm_sb")
            qkT_sb = pp.tile([128, 4, S], bf16, name="qkT_sb")
            rsq_sb = pp.tile([64, 4, S], fp32, name="rsq_sb")
            rd_sb = pp.tile([32, S], fp32, name="rd_sb")
            qdup_sb = pp.tile([128, 4, S], bf16, name="qdup_sb")
            kdup_sb = pp.tile([128, 4, 8, 128], bf16, name="kdup_sb")
            v_sb = pp.tile([128, 16, 4 * 128], bf16, name="v_sb")
            oT_sb = pp.tile([128, 2, S], bf16, name="oT_sb")

            nc.sync.dma_start(xTa_sb[:], xTa.rearrange("(c p) s -> p c s", p=128))
            nc.sync.dma_start(wqk_sb[:], wqk.rearrange("(c p) m -> p c m", p=128))
            nc.sync.dma_start(wv_sb[:], wv.rearrange("(c p) m -> p c m", p=128))
            nc.sync.dma_start(wpr_sb[:], wpr.rearrange("(c p) m -> p c m", p=128))
            nc.sync.dma_start(cos_sb[:], cosT2[:])
            nc.sync.dma_start(sin_sb[:], sinT2[:])
            nc.sync.dma_start(mask_sb[:], mask33[:])
            nc.sync.dma_start(ones4_sb[:], ones4[:])
            nc.sync.dma_start(perm_sb[:], perm[:])
            nc.vector.memset(rd_sb[:], 0.0)
            nc.vector.memset(rsq_sb[:], 0.0)
            # ones half-blocks of v (cols 64-127 of each 128-stride head
            # block) are constant: set once, outside the rep loop.  64 ones
            # columns replicate the softmax denominator into psum partitions
            # 64-127 of the attn@v accumulator, so the normalize tail needs no
            # separate broadcast.
            nc.vector.memset(
                v_sb[:].rearrange("p s (h c) -> p s h c", h=4)[:, :, :, 64:128], 1.0
            )

            def _phases():
                # ---------- Phase B+D: QKV projections + rmsnorm + RoPE ----
                with (
                    tc.tile_pool(name="qkps", bufs=2, space="PSUM") as qkps,
                    tc.tile_pool(name="vsps", bufs=1, space="PSUM") as vsps,
                    tc.tile_pool(name="ssps", bufs=1, space="PSUM") as ssps,
                    tc.tile_pool(name="ropeps", bufs=1, space="PSUM") as ropeps,
                    tc.tile_pool(name="cqps", bufs=1, space="PSUM") as cqps,
                    tc.tile_pool(name="sqpool", bufs=3) as sqpool,
                    tc.tile_pool(name="ropetmp", bufs=2) as ropetmp,
                ):
                    def b_block(m):
                        # q,k transposed: psum[m-chunk, seg].  The rmsnorm
                        # reduction (ss-matmul + ln/exp) for segment k is
                        # emitted after segment k+1's matmul group so the PE
                        # never waits on the DVE square.
                        deferred = []

                        def norm_tail(seg):
                            sq = sqpool.tile([128, 512], bf16, tag="sq")
                            qsl = qkT_sb[:, m, seg * 512 : (seg + 1) * 512]
                            nc.vector.tensor_mul(out=sq[:], in0=qsl, in1=qsl)
                            ss = ssps.tile([33, 512], fp32, tag="ss")
                            nc.tensor.matmul(
                                ss[:], mask_sb[:], sq[:], start=True, stop=True
                            )
                            # rsqrt(mean square) as exp(-0.5*ln) (AF.Rsqrt is
                            # blocked in bass); eps negligible for unit-normal
                            lt = sqpool.tile([33, 512], fp32, tag="lt")
                            nc.scalar.activation(lt[:], ss[:], AF.Ln, scale=1.0 / HD)
                            nc.scalar.activation(
                                rsq_sb[0:33, m, seg * 512 : (seg + 1) * 512],
                                lt[:],
                                AF.Exp,
                                scale=-0.5,
                            )

                        for seg in range(NSEG):
                            ps = qkps.tile([128, 512], fp32, tag="qk")
                            for kk in range(9):
                                nc.tensor.matmul(
                                    ps[:],
                                    wqk_sb[:, kk, m * 128 : (m + 1) * 128],
                                    xTa_sb[:, kk, seg * 512 : (seg + 1) * 512],
                                    start=(kk == 0),
                                    stop=(kk == 8),
                                )
                            nc.vector.tensor_copy(
                                out=qkT_sb[:, m, seg * 512 : (seg + 1) * 512],
                                in_=ps[:],
                            )
                            for fn in deferred:
                                fn()
                            deferred = [lambda seg=seg: norm_tail(seg)]
                        for fn in deferred:
                            fn()

                    def v_block(si):
                        ps = vsps.tile([128, 256], fp32, tag="vps")
                        for kk in range(9):
                            nc.tensor.matmul(
                                ps[:],
                                xTa_sb[:, kk, si * 128 : (si + 1) * 128],
                                wv_sb[:, kk, :],
                                start=(kk == 0),
                                stop=(kk == 8),
                            )
                        nc.vector.tensor_copy(
                            out=v_sb[:, si].rearrange("p (h c) -> p h c", h=4)[
                                :, :, 0:64
                            ],
                            in_=ps[:].rearrange("p (h c) -> p h c", h=4),
                        )

                    def d_block(m):
                        # norm-scale then rotate:  qk = rope(qk * cq)
                        for ch in range(2):
                            c0 = ch * 1024
                            sl = (slice(None), m, slice(c0, c0 + 1024))
                            cq = cqps.tile([128, 1024], fp32, tag="cq")
                            for hf in range(2):
                                r = 32 * hf
                                for s2 in range(2):
                                    nc.tensor.matmul(
                                        cq[
                                            64 * hf : 64 * hf + 64,
                                            s2 * 512 : (s2 + 1) * 512,
                                        ],
                                        ones4_sb[r : r + 32, 0:64],
                                        rsq_sb[
                                            r : r + 32,
                                            m,
                                            c0 + s2 * 512 : c0 + (s2 + 1) * 512,
                                        ],
                                        start=True,
                                        stop=True,
                                        tile_position=(r, 64 * hf),
                                    )
                            nc.vector.tensor_mul(
                                out=qkT_sb[sl], in0=qkT_sb[sl], in1=cq[:]
                            )
                            qs = ropeps.tile([128, 1024], fp32, tag="qs")
                            for s2 in range(2):
                                nc.tensor.matmul(
                                    qs[:, s2 * 512 : (s2 + 1) * 512],
                                    perm_sb[:],
                                    qkT_sb[:, m, c0 + s2 * 512 : c0 + (s2 + 1) * 512],
                                    start=True,
                                    stop=True,
                                )
                            t1 = ropetmp.tile([128, 1024], bf16, tag="t1")
                            nc.vector.tensor_mul(
                                out=t1[:], in0=qkT_sb[sl], in1=cos_sb[:, c0 : c0 + 1024]
                            )
                            t2 = ropetmp.tile([128, 1024], bf16, tag="t2")
                            nc.vector.tensor_mul(
                                out=t2[:], in0=qs[:], in1=sin_sb[:, c0 : c0 + 1024]
                            )
                            nc.vector.tensor_add(out=qkT_sb[sl], in0=t1[:], in1=t2[:])

                    def dup_block(h):
                        p, hf = h // 2, h % 2
                        qsrc = qkT_sb[64 * hf : 64 * hf + 64, p, :]
                        nc.sync.dma_start(qdup_sb[0:64, h, :], qsrc)
                        nc.sync.dma_start(qdup_sb[64:128, h, :], qsrc)
                        ksrc = qkT_sb[64 * hf : 64 * hf + 64, 2 + p, :].rearrange(
                            "p (j two s) -> p j two s", two=2, s=128
                        )
                        nc.sync.dma_start(kdup_sb[0:64, h, :, :], ksrc[:, :, 0, :])
                        nc.sync.dma_start(kdup_sb[64:128, h, :, :], ksrc[:, :, 1, :])

                    # order: critical path first (pair 0 = heads 0,1 needs m=0,2)
                    b_block(0)
                    b_block(2)
                    d_block(0)
                    d_block(2)
                    dup_block(0)
                    dup_block(1)
                    for si in range(16):
                        v_block(si)
                    b_block(1)
                    b_block(3)
                    d_block(1)
                    d_block(3)
                    dup_block(2)
                    dup_block(3)

                if stage >= "E":
                    # ---------- Phase E: attention ----------
                    with (
                        tc.tile_pool(name="scps", bufs=3, space="PSUM") as scps,
                        tc.tile_pool(name="otps", bufs=1, space="PSUM") as otps,
                        tc.tile_pool(name="expool", bufs=4) as expool,
                        tc.tile_pool(name="rbpool", bufs=2) as rbpool,
                    ):
                        pending_tail = [None]

                        def e_block(h, qc):
                            p, hf = h // 2, h % 2
                            q0 = qc * 1024
                            oT = otps.tile([128, 1024], fp32, tag="ot", name="oT")

                            def scores_exps(jp):
                                sc = [
                                    scps.tile(
                                        [128, 1024], fp32, tag="sc", name=f"sc{g}"
                                    )
                                    for g in range(2)
                                ]
                                exs = []
                                for s2 in range(2):
                                    for g in range(2):
                                        r = 64 * g
                                        nc.tensor.matmul(
                                            sc[g][:, s2 * 512 : (s2 + 1) * 512],
                                            kdup_sb[r : r + 64, h, jp, :],
                                            qdup_sb[
                                                r : r + 64,
                                                h,
                                                q0 + s2 * 512 : q0 + (s2 + 1) * 512,
                                            ],
                                            start=True,
                                            stop=True,
                                        )
                                for g in range(2):
                                    ex = expool.tile([128, 1024], bf16, tag="ex")
                                    nc.scalar.activation(
                                        ex[:], sc[g][:], AF.Exp, scale=0.125
                                    )
                                    exs.append(ex)
                                return exs

                            def avs(jp, exs):
                                for g in range(2):
                                    cchunk = 2 * jp + g
                                    for s2 in range(2):
                                        nc.tensor.matmul(
                                            oT[:, s2 * 512 : (s2 + 1) * 512],
                                            v_sb[:, cchunk, h * 128 : h * 128 + 128],
                                            exs[g][:, s2 * 512 : (s2 + 1) * 512],
                                            start=(jp == 0 and g == 0),
                                            stop=(jp == 7 and g == 1),
                                        )

                            prev_exs = scores_exps(0)
                            for jp in range(1, 8):
                                exs = scores_exps(jp)
                                avs(jp - 1, prev_exs)
                                prev_exs = exs
                                if jp == 1 and pending_tail[0] is not None:
                                    pending_tail[0]()
                                    pending_tail[0] = None
                            avs(7, prev_exs)

                            def tail():
                                # rows 64-127 of oT all hold the denominator
                                # (64 ones-columns in v): recip a 64-row copy,
                                # then one multiply moving psum -> SBUF
                                dn = rbpool.tile([64, 1024], fp32, tag="dn")
                                nc.vector.tensor_copy(out=dn[:], in_=oT[64:128, :])
                                rb = rbpool.tile([64, 1024], fp32, tag="rb")
                                nc.vector.reciprocal_approx_fast(
                                    out=rb[:], in_=dn[:]
                                )
                                nc.vector.tensor_mul(
                                    out=oT_sb[
                                        64 * hf : 64 * hf + 64, p, q0 : q0 + 1024
                                    ],
                                    in0=oT[0:64, :],
                                    in1=rb[:],
                                )

                            pending_tail[0] = tail

                        for h in range(4):
                            for qc in range(2):
                                e_block(h, qc)
                        pending_tail[0]()

                if stage >= "F":
                    # ---------- Phase F: output projection ----------
                    with (
                        tc.tile_pool(name="prps", bufs=2, space="PSUM") as prps,
                        tc.tile_pool(name="outpool", bufs=3) as outpool,
                    ):
                        for si in range(16):
                            ob = outpool.tile([128, D], fp32, tag="ob")
                            for ncol in range(2):
                                ps = prps.tile([128, 512], fp32, tag="pr")
                                for kc in range(2):
                                    nc.tensor.matmul(
                                        ps[:],
                                        oT_sb[:, kc, si * 128 : (si + 1) * 128],
                                        wpr_sb[:, kc, ncol * 512 : (ncol + 1) * 512],
                                        start=(kc == 0),
                                        stop=(kc == 1),
                                    )
                                nc.vector.tensor_copy(
                                    out=ob[:, ncol * 512 : (ncol + 1) * 512], in_=ps[:]
                                )
                            nc.sync.dma_start(out[si * 128 : (si + 1) * 128, :], ob[:])

            for _rep in range(reps):
                _phases()

            if debug:
                for sb, dram in (
                    (qkT_sb, dbg_qkT),
                    (rsq_sb, dbg_rsq),
                    (qdup_sb, dbg_qdup),
                    (kdup_sb, dbg_kdup),
                    (v_sb, dbg_v),
                    (oT_sb, dbg_oT),
                    (rd_sb, dbg_rd),
                ):
                    nc.sync.dma_start(dram[...], sb[:])

            if stage < "F":
                with tc.tile_pool(name="zo", bufs=1) as zp:
                    zt = zp.tile([128, D], fp32, name="zt")
                    nc.vector.memset(zt[:], 0.0)
                    nc.sync.dma_start(out[0:128, :], zt[:])

    nc.finalize()
    return nc


def _host_inputs(x, Wqkv, bqkv, qg, kg, Wproj, cos, sin):
    """Build the 8 per-core input maps (numpy, host-side sharding/layout)."""
    S_, D_ = S, D
    x = np.asarray(x, dtype=np.float32)
    Wqkv = np.asarray(Wqkv, dtype=np.float32)
    bqkv = np.asarray(bqkv, dtype=np.float32)
    qg = np.asarray(qg, dtype=np.float32)
    kg = np.asarray(kg, dtype=np.float32)
    Wproj = np.asarray(Wproj, dtype=np.float32)
    cos = np.asarray(cos, dtype=np.float32)
    sin = np.asarray(sin, dtype=np.float32)

    # shared tensors
    cosT2 = np.concatenate([cos.T, cos.T], axis=0).astype(BF16)  # [128, S]
    sf = np.concatenate([-sin[:, : HD // 2], sin[:, HD // 2 :]], axis=1)
    sinT2 = np.concatenate([sf.T, sf.T], axis=0).astype(BF16)  # [128, S]
    mask33 = np.zeros((128, 33), dtype=BF16)
    mask33[0:64, 0:32] = 1  # cols 1-31 duplicate col 0: keeps unused rows finite
    mask33[64:128, 32] = 1
    ones4 = np.zeros((128, 64), dtype=np.float32)
    ones4[0, :] = 1.0
    ones4[32, :] = 1.0
    permm = np.zeros((128, 128), dtype=BF16)
    for mcol in range(128):
        rot = (mcol + 32) % 64 + 64 * (mcol // 64)
        permm[rot, mcol] = 1.0

    qg4 = np.tile(qg, HPC)  # [256]
    kg4 = np.tile(kg, HPC)

    xTa_b = []
    for b in range(B):
        t = np.zeros((1152, S_), dtype=BF16)
        t[0:D_, :] = x[b].T.astype(BF16)
        t[D_, :] = 1.0
        xTa_b.append(t)

    in_maps = []
    for core in range(N_CORES):
        b = core // 4
        hg = core % 4
        cq0 = hg * CW
        xTa = xTa_b[b]

        wqk = np.zeros((1152, 2 * CW), dtype=np.float32)
        wqk[0:D_, 0:CW] = Wqkv[:, cq0 : cq0 + CW] * qg4[None, :]
        wqk[D_, 0:CW] = bqkv[cq0 : cq0 + CW] * qg4
        wqk[0:D_, CW:] = Wqkv[:, D_ + cq0 : D_ + cq0 + CW] * kg4[None, :]
        wqk[D_, CW:] = bqkv[D_ + cq0 : D_ + cq0 + CW] * kg4
        wqk = wqk.astype(BF16)

        wv = np.zeros((1152, CW), dtype=np.float32)
        wv[0:D_, :] = Wqkv[:, 2 * D_ + cq0 : 2 * D_ + cq0 + CW]
        wv[D_, :] = bqkv[2 * D_ + cq0 : 2 * D_ + cq0 + CW]
        wv = wv.astype(BF16)

        wpr = Wproj[cq0 : cq0 + CW, :].astype(BF16)

        in_maps.append(
            {
                "xTa": xTa,
                "wqk": wqk,
                "wv": wv,
                "wpr": np.ascontiguousarray(wpr),
                "cosT2": cosT2,
                "sinT2": sinT2,
                "mask33": mask33,
                "ones4": ones4,
                "perm": permm,
            }
        )
    return in_maps


_NC_CACHE = None


def kernel(x, Wqkv, bqkv, qg, kg, Wproj, bproj, cos, sin):
    global LAST_RESULTS, _NC_CACHE
    from concourse.bass_utils import run_bass_kernel_spmd

    if _NC_CACHE is None:
        _NC_CACHE = _build_bass()
    nc = _NC_CACHE

    in_maps = _host_inputs(x, Wqkv, bqkv, qg, kg, Wproj, cos, sin)
    res = run_bass_kernel_spmd(nc, in_maps, core_ids=list(range(N_CORES)))
    LAST_RESULTS = res

    bproj = np.asarray(bproj, dtype=np.float32)
    out = np.zeros((B, S, D), dtype=np.float32)
    for b in range(B):
        acc = np.zeros((S, D), dtype=np.float32)
        for i in range(4):
            acc += res.results[4 * b + i]["out"]
        out[b] = acc + bproj[None, :]
    return out


# revision 15
# speedup vs baseline: 1.0903x; 1.0903x over previous
"""Multi-head attention block (QKV proj + RMSNorm + RoPE + SDPA + out proj)
sharded across 8 Trainium2 NeuronCores.

Sharding: data-parallel over batch (B=2 -> 2 groups of 4 cores), tensor-parallel
over heads (16 heads -> 4 heads/core).  Each core computes a partial output
projection for its 4 heads; the host sums the 4 partials per batch and adds
bproj.

Device layout strategy (per core, batch b, heads hg*4..hg*4+3):
  - qkv computed transposed (d-on-partitions) so scores/attn matmuls need no
    on-chip transposes:  qT,kT [4*64, 2048],  v natural [2048, 4*64 | ones].
  - RMSNorm: sum-of-squares via mask-matmul (partition reduction on PE, sums
    at psum partitions {0,32}), rsqrt as ln+exp(-0.5*ln) on ACT in [33,512]
    blocks.  The rsqrt rows are broadcast to 64 partitions by a ones-matmul
    and applied with one DVE multiply BEFORE RoPE (the norm scale is constant
    within a head at fixed position, so it commutes with the rotation).
  - RoPE half-rotation via a signed permutation matmul on PE.
  - Attention processes ONE head at a time with k-chunks PAIRED across PE
    row-groups (kdup holds even chunks at partitions 0-63, odd at 64-127;
    qdup duplicates q to both halves).  Each pair: 4 concurrent score
    matmuls -> 2 [128,1024] exps (ACT) -> 4 attn@v matmuls accumulating in
    one [65,1024] psum tile.  PSUM: 3 score slots (6 banks) + 1 accumulator
    (2 banks) = 8 banks, letting scores for pair j+1 overlap exp of pair j
    so the phase runs at the ACT exp roofline.
  - softmax denominator comes pre-broadcast from a 64-wide ones block in
    V (psum rows 64-127 of the [128, q] attn@v accumulator all hold the
    denominator); 1/denom via the fast custom-DVE reciprocal, then one
    multiply normalizes and moves psum -> SBUF.  The tail is emitted after
    the next block's first score pairs so ACT never idles at boundaries.
"""

import numpy as np
import ml_dtypes

B, S, D, H = 2, 2048, 1024, 16
HD = D // H
EPS = 1e-6
N_CORES = 8
HPC = H // 4  # heads per core = 4
CW = HPC * HD  # per-core head-col width = 256

BF16 = ml_dtypes.bfloat16

LAST_RESULTS = None  # stash of BassKernelResults for test harness introspection


def _build_bass(debug=False, stage="F", reps=1):
    """stage: progressively include phases: 'B' < 'D' < 'E' < 'F'.
    reps>1 repeats the compute phases (benchmarking: marginal time per rep
    = steady-state compute time, input DMAs amortized)."""
    import concourse.mybir as mybir
    import concourse.tile as tile
    from concourse import bacc

    fp32 = mybir.dt.float32
    bf16 = mybir.dt.bfloat16
    AF = mybir.ActivationFunctionType

    nc = bacc.Bacc()

    # ---- DRAM I/O ----
    xTa = nc.dram_tensor("xTa", [1152, S], bf16, kind="ExternalInput")
    wqk = nc.dram_tensor("wqk", [1152, 2 * CW], bf16, kind="ExternalInput")
    wv = nc.dram_tensor("wv", [1152, CW], bf16, kind="ExternalInput")
    wpr = nc.dram_tensor("wpr", [CW, D], bf16, kind="ExternalInput")
    cosT2 = nc.dram_tensor("cosT2", [128, S], bf16, kind="ExternalInput")
    sinT2 = nc.dram_tensor("sinT2", [128, S], bf16, kind="ExternalInput")
    mask33 = nc.dram_tensor("mask33", [128, 33], bf16, kind="ExternalInput")
    ones4 = nc.dram_tensor("ones4", [128, 64], fp32, kind="ExternalInput")
    perm = nc.dram_tensor("perm", [128, 128], bf16, kind="ExternalInput")
    out = nc.dram_tensor("out", [S, D], fp32, kind="ExternalOutput")
    if debug:
        dbg_qkT = nc.dram_tensor("dbg_qkT", [128, 4, S], bf16, kind="ExternalOutput")
        dbg_rsq = nc.dram_tensor("dbg_rsq", [64, 4, S], fp32, kind="ExternalOutput")
        dbg_qdup = nc.dram_tensor("dbg_qdup", [128, 4, S], bf16, kind="ExternalOutput")
        dbg_kdup = nc.dram_tensor("dbg_kdup", [128, 4, 8, 128], bf16, kind="ExternalOutput")
        dbg_v = nc.dram_tensor("dbg_v", [128, 16, 264], bf16, kind="ExternalOutput")
        dbg_oT = nc.dram_tensor("dbg_oT", [128, 2, S], bf16, kind="ExternalOutput")
        dbg_rd = nc.dram_tensor("dbg_rd", [32, S], fp32, kind="ExternalOutput")

    NSEG = S // 512  # 4

    with tile.TileContext(nc) as tc:
        with tc.tile_pool(name="persist", bufs=1) as pp:
            # persistent SBUF tensors
            xTa_sb = pp.tile([128, 9, S], bf16, name="xTa_sb")
            wqk_sb = pp.tile([128, 9, 2 * CW], bf16, name="wqk_sb")
            wv_sb = pp.tile([128, 9, CW], bf16, name="wv_sb")
            wpr_sb = pp.tile([128, 2, D], bf16, name="wpr_sb")
            cos_sb = pp.tile([128, S], bf16, name="cos_sb")
            sin_sb = pp.tile([128, S], bf16, name="sin_sb")
            mask_sb = pp.tile([128, 33], bf16, name="mask_sb")
            ones4_sb = pp.tile([128, 64], fp32, name="ones4_sb")
            perm_sb = pp.tile([128, 128], bf16, name="perm_sb")
            qkT_sb = pp.tile([128, 4, S], bf16, name="qkT_sb")
            rsq_sb = pp.tile([64, 4, S], fp32, name="rsq_sb")
            qdup_sb = pp.tile([128, 4, S], bf16, name="qdup_sb")
            kdup_sb = pp.tile([128, 4, 8, 128], bf16, name="kdup_sb")
            v_sb = pp.tile([128, 16, 4 * 128], bf16, name="v_sb")
            oT_sb = pp.tile([128, 2, S], bf16, name="oT_sb")

            nc.sync.dma_start(xTa_sb[:], xTa.rearrange("(c p) s -> p c s", p=128))
            nc.sync.dma_start(wqk_sb[:], wqk.rearrange("(c p) m -> p c m", p=128))
            nc.sync.dma_start(wv_sb[:], wv.rearrange("(c p) m -> p c m", p=128))
            nc.sync.dma_start(wpr_sb[:], wpr.rearrange("(c p) m -> p c m", p=128))
            nc.sync.dma_start(cos_sb[:], cosT2[:])
            nc.sync.dma_start(sin_sb[:], sinT2[:])
            nc.sync.dma_start(mask_sb[:], mask33[:])
            nc.sync.dma_start(ones4_sb[:], ones4[:])
            nc.sync.dma_start(perm_sb[:], perm[:])
            nc.vector.memset(rsq_sb[:], 0.0)
            # ones half-blocks of v (cols 64-127 of each 128-stride head
            # block) are constant: set once, outside the rep loop.  64 ones
            # columns replicate the softmax denominator into psum partitions
            # 64-127 of the attn@v accumulator, so the normalize tail needs no
            # separate broadcast.
            nc.vector.memset(
                v_sb[:].rearrange("p s (h c) -> p s h c", h=4)[:, :, :, 64:128], 1.0
            )

            def _phases():
                # ---------- Phase B+D: QKV projections + rmsnorm + RoPE ----
                with (
                    tc.tile_pool(name="qkps", bufs=2, space="PSUM") as qkps,
                    tc.tile_pool(name="vsps", bufs=1, space="PSUM") as vsps,
                    tc.tile_pool(name="ssps", bufs=1, space="PSUM") as ssps,
                    tc.tile_pool(name="ropeps", bufs=1, space="PSUM") as ropeps,
                    tc.tile_pool(name="cqps", bufs=1, space="PSUM") as cqps,
                    tc.tile_pool(name="sqpool", bufs=3) as sqpool,
                    tc.tile_pool(name="ropetmp", bufs=2) as ropetmp,
                ):
                    def b_block(m):
                        # q,k transposed: psum[m-chunk, seg].  The rmsnorm
                        # reduction (ss-matmul + ln/exp) for segment k is
                        # emitted after segment k+1's matmul group so the PE
                        # never waits on the DVE square.
                        deferred = []

                        def norm_tail(seg):
                            sq = sqpool.tile([128, 512], bf16, tag="sq")
                            qsl = qkT_sb[:, m, seg * 512 : (seg + 1) * 512]
                            nc.vector.tensor_mul(out=sq[:], in0=qsl, in1=qsl)
                            ss = ssps.tile([33, 512], fp32, tag="ss")
                            nc.tensor.matmul(
                                ss[:], mask_sb[:], sq[:], start=True, stop=True
                            )
                            # rsqrt(mean square) as exp(-0.5*ln) (AF.Rsqrt is
                            # blocked in bass); eps negligible for unit-normal
                            lt = sqpool.tile([33, 512], fp32, tag="lt")
                            nc.scalar.activation(lt[:], ss[:], AF.Ln, scale=1.0 / HD)
                            nc.scalar.activation(
                                rsq_sb[0:33, m, seg * 512 : (seg + 1) * 512],
                                lt[:],
                                AF.Exp,
                                scale=-0.5,
                            )

                        for seg in range(NSEG):
                            ps = qkps.tile([128, 512], fp32, tag="qk")
                            for kk in range(9):
                                nc.tensor.matmul(
                                    ps[:],
                                    wqk_sb[:, kk, m * 128 : (m + 1) * 128],
                                    xTa_sb[:, kk, seg * 512 : (seg + 1) * 512],
                                    start=(kk == 0),
                                    stop=(kk == 8),
                                )
                            nc.vector.tensor_copy(
                                out=qkT_sb[:, m, seg * 512 : (seg + 1) * 512],
                                in_=ps[:],
                            )
                            for fn in deferred:
                                fn()
                            deferred = [lambda seg=seg: norm_tail(seg)]
                        for fn in deferred:
                            fn()

                    def v_block(si):
                        ps = vsps.tile([128, 256], fp32, tag="vps")
                        for kk in range(9):
                            nc.tensor.matmul(
                                ps[:],
                                xTa_sb[:, kk, si * 128 : (si + 1) * 128],
                                wv_sb[:, kk, :],
                                start=(kk == 0),
                                stop=(kk == 8),
                            )
                        nc.vector.tensor_copy(
                            out=v_sb[:, si].rearrange("p (h c) -> p h c", h=4)[
                                :, :, 0:64
                            ],
                            in_=ps[:].rearrange("p (h c) -> p h c", h=4),
                        )

                    def d_block(m):
                        # norm-scale then rotate:  qk = rope(qk * cq)
                        for ch in range(2):
                            c0 = ch * 1024
                            sl = (slice(None), m, slice(c0, c0 + 1024))
                            cq = cqps.tile([128, 1024], fp32, tag="cq")
                            for hf in range(2):
                                r = 32 * hf
                                for s2 in range(2):
                                    nc.tensor.matmul(
                                        cq[
                                            64 * hf : 64 * hf + 64,
                                            s2 * 512 : (s2 + 1) * 512,
                                        ],
                                        ones4_sb[r : r + 32, 0:64],
                                        rsq_sb[
                                            r : r + 32,
                                            m,
                                            c0 + s2 * 512 : c0 + (s2 + 1) * 512,
                                        ],
                                        start=True,
                                        stop=True,
                                        tile_position=(r, 64 * hf),
                                    )
                            nc.vector.tensor_mul(
                                out=qkT_sb[sl], in0=qkT_sb[sl], in1=cq[:]
                            )
                            qs = ropeps.tile([128, 1024], fp32, tag="qs")
                            for s2 in range(2):
                                nc.tensor.matmul(
                                    qs[:, s2 * 512 : (s2 + 1) * 512],
                                    perm_sb[:],
                                    qkT_sb[:, m, c0 + s2 * 512 : c0 + (s2 + 1) * 512],
                                    start=True,
                                    stop=True,
                                )
                            t1 = ropetmp.tile([128, 1024], bf16, tag="t1")
                            nc.vector.tensor_mul(
                                out=t1[:], in0=qkT_sb[sl], in1=cos_sb[:, c0 : c0 + 1024]
                            )
                            t2 = ropetmp.tile([128, 1024], bf16, tag="t2")
                            nc.vector.tensor_mul(
                                out=t2[:], in0=qs[:], in1=sin_sb[:, c0 : c0 + 1024]
                            )
                            nc.vector.tensor_add(out=qkT_sb[sl], in0=t1[:], in1=t2[:])

                    def dup_block(h):
                        p, hf = h // 2, h % 2
                        qsrc = qkT_sb[64 * hf : 64 * hf + 64, p, :]
                        nc.sync.dma_start(qdup_sb[0:64, h, :], qsrc)
                        nc.sync.dma_start(qdup_sb[64:128, h, :], qsrc)
                        ksrc = qkT_sb[64 * hf : 64 * hf + 64, 2 + p, :].rearrange(
                            "p (j two s) -> p j two s", two=2, s=128
                        )
                        nc.sync.dma_start(kdup_sb[0:64, h, :, :], ksrc[:, :, 0, :])
                        nc.sync.dma_start(kdup_sb[64:128, h, :, :], ksrc[:, :, 1, :])

                    # order: critical path first (pair 0 = heads 0,1 needs
                    # m=0,2).  d/dup blocks are emitted right after their
                    # producers so the readiness scheduler can interleave
                    # their chains with later b/v matmul streams.
                    b_block(0)
                    d_block(0)
                    b_block(2)
                    d_block(2)
                    dup_block(0)
                    dup_block(1)
                    for si in range(8):
                        v_block(si)
                    b_block(1)
                    d_block(1)
                    b_block(3)
                    d_block(3)
                    dup_block(2)
                    dup_block(3)
                    for si in range(8, 16):
                        v_block(si)

                if stage >= "E":
                    # ---------- Phase E: attention ----------
                    with (
                        tc.tile_pool(name="scps", bufs=3, space="PSUM") as scps,
                        tc.tile_pool(name="otps", bufs=1, space="PSUM") as otps,
                        tc.tile_pool(name="expool", bufs=4) as expool,
                        tc.tile_pool(name="rbpool", bufs=2) as rbpool,
                    ):
                        pending_tail = [None]

                        def e_block(h, qc):
                            p, hf = h // 2, h % 2
                            q0 = qc * 1024
                            oT = otps.tile([128, 1024], fp32, tag="ot", name="oT")

                            def scores_exps(jp):
                                sc = [
                                    scps.tile(
                                        [128, 1024], fp32, tag="sc", name=f"sc{g}"
                                    )
                                    for g in range(2)
                                ]
                                exs = []
                                for s2 in range(2):
                                    for g in range(2):
                                        r = 64 * g
                                        nc.tensor.matmul(
                                            sc[g][:, s2 * 512 : (s2 + 1) * 512],
                                            kdup_sb[r : r + 64, h, jp, :],
                                            qdup_sb[
                                                r : r + 64,
                                                h,
                                                q0 + s2 * 512 : q0 + (s2 + 1) * 512,
                                            ],
                                            start=True,
                                            stop=True,
                                        )
                                for g in range(2):
                                    ex = expool.tile([128, 1024], bf16, tag="ex")
                                    nc.scalar.activation(
                                        ex[:], sc[g][:], AF.Exp, scale=0.125
                                    )
                                    exs.append(ex)
                                return exs

                            def avs(jp, exs):
                                for g in range(2):
                                    cchunk = 2 * jp + g
                                    for s2 in range(2):
                                        nc.tensor.matmul(
                                            oT[:, s2 * 512 : (s2 + 1) * 512],
                                            v_sb[:, cchunk, h * 128 : h * 128 + 128],
                                            exs[g][:, s2 * 512 : (s2 + 1) * 512],
                                            start=(jp == 0 and g == 0),
                                            stop=(jp == 7 and g == 1),
                                        )

                            prev_exs = scores_exps(0)
                            for jp in range(1, 8):
                                exs = scores_exps(jp)
                                avs(jp - 1, prev_exs)
                                prev_exs = exs
                                if jp == 1 and pending_tail[0] is not None:
                                    pending_tail[0]()
                                    pending_tail[0] = None
                            avs(7, prev_exs)

                            def tail():
                                # rows 64-127 of oT all hold the denominator
                                # (64 ones-columns in v): recip a 64-row copy,
                                # then one multiply moving psum -> SBUF
                                dn = rbpool.tile([64, 1024], fp32, tag="dn")
                                nc.vector.tensor_copy(out=dn[:], in_=oT[64:128, :])
                                rb = rbpool.tile([64, 1024], fp32, tag="rb")
                                nc.vector.reciprocal_approx_fast(
                                    out=rb[:], in_=dn[:]
                                )
                                nc.vector.tensor_mul(
                                    out=oT_sb[
                                        64 * hf : 64 * hf + 64, p, q0 : q0 + 1024
                                    ],
                                    in0=oT[0:64, :],
                                    in1=rb[:],
                                )

                            pending_tail[0] = tail

                        for h in range(4):
                            for qc in range(2):
                                e_block(h, qc)
                        pending_tail[0]()

                if stage >= "F":
                    # ---------- Phase F: output projection ----------
                    with (
                        tc.tile_pool(name="prps", bufs=2, space="PSUM") as prps,
                        tc.tile_pool(name="outpool", bufs=3) as outpool,
                    ):
                        for si in range(16):
                            ob = outpool.tile([128, D], fp32, tag="ob")
                            for ncol in range(2):
                                ps = prps.tile([128, 512], fp32, tag="pr")
                                for kc in range(2):
                                    nc.tensor.matmul(
                                        ps[:],
                                        oT_sb[:, kc, si * 128 : (si + 1) * 128],
                                        wpr_sb[:, kc, ncol * 512 : (ncol + 1) * 512],
                                        start=(kc == 0),
                                        stop=(kc == 1),
                                    )
                                osl = ob[:, ncol * 512 : (ncol + 1) * 512]
                                if ncol == 0:
                                    nc.vector.tensor_copy(out=osl, in_=ps[:])
                                else:
                                    nc.scalar.activation(osl, ps[:], AF.Copy)
                            nc.sync.dma_start(out[si * 128 : (si + 1) * 128, :], ob[:])

            for _rep in range(reps):
                _phases()

            if debug:
                for sb, dram in (
                    (qkT_sb, dbg_qkT),
                    (rsq_sb, dbg_rsq),
                    (qdup_sb, dbg_qdup),
                    (kdup_sb, dbg_kdup),
                    (v_sb, dbg_v),
                    (oT_sb, dbg_oT),
                    (rd_sb, dbg_rd),
                ):
                    nc.sync.dma_start(dram[...], sb[:])

            if stage < "F":
                with tc.tile_pool(name="zo", bufs=1) as zp:
                    zt = zp.tile([128, D], fp32, name="zt")
                    nc.vector.memset(zt[:], 0.0)
                    nc.sync.dma_start(out[0:128, :], zt[:])

    nc.finalize()
    return nc


def _host_inputs(x, Wqkv, bqkv, qg, kg, Wproj, cos, sin):
    """Build the 8 per-core input maps (numpy, host-side sharding/layout)."""
    S_, D_ = S, D
    x = np.asarray(x, dtype=np.float32)
    Wqkv = np.asarray(Wqkv, dtype=np.float32)
    bqkv = np.asarray(bqkv, dtype=np.float32)
    qg = np.asarray(qg, dtype=np.float32)
    kg = np.asarray(kg, dtype=np.float32)
    Wproj = np.asarray(Wproj, dtype=np.float32)
    cos = np.asarray(cos, dtype=np.float32)
    sin = np.asarray(sin, dtype=np.float32)

    # shared tensors
    cosT2 = np.concatenate([cos.T, cos.T], axis=0).astype(BF16)  # [128, S]
    sf = np.concatenate([-sin[:, : HD // 2], sin[:, HD // 2 :]], axis=1)
    sinT2 = np.concatenate([sf.T, sf.T], axis=0).astype(BF16)  # [128, S]
    mask33 = np.zeros((128, 33), dtype=BF16)
    mask33[0:64, 0:32] = 1  # cols 1-31 duplicate col 0: keeps unused rows finite
    mask33[64:128, 32] = 1
    ones4 = np.zeros((128, 64), dtype=np.float32)
    ones4[0, :] = 1.0
    ones4[32, :] = 1.0
    permm = np.zeros((128, 128), dtype=BF16)
    for mcol in range(128):
        rot = (mcol + 32) % 64 + 64 * (mcol // 64)
        permm[rot, mcol] = 1.0

    qg4 = np.tile(qg, HPC)  # [256]
    kg4 = np.tile(kg, HPC)

    xTa_b = []
    for b in range(B):
        t = np.zeros((1152, S_), dtype=BF16)
        t[0:D_, :] = x[b].T.astype(BF16)
        t[D_, :] = 1.0
        xTa_b.append(t)

    in_maps = []
    for core in range(N_CORES):
        b = core // 4
        hg = core % 4
        cq0 = hg * CW
        xTa = xTa_b[b]

        wqk = np.zeros((1152, 2 * CW), dtype=np.float32)
        wqk[0:D_, 0:CW] = Wqkv[:, cq0 : cq0 + CW] * qg4[None, :]
        wqk[D_, 0:CW] = bqkv[cq0 : cq0 + CW] * qg4
        wqk[0:D_, CW:] = Wqkv[:, D_ + cq0 : D_ + cq0 + CW] * kg4[None, :]
        wqk[D_, CW:] = bqkv[D_ + cq0 : D_ + cq0 + CW] * kg4
        wqk = wqk.astype(BF16)

        wv = np.zeros((1152, CW), dtype=np.float32)
        wv[0:D_, :] = Wqkv[:, 2 * D_ + cq0 : 2 * D_ + cq0 + CW]
        wv[D_, :] = bqkv[2 * D_ + cq0 : 2 * D_ + cq0 + CW]
        wv = wv.astype(BF16)

        wpr = Wproj[cq0 : cq0 + CW, :].astype(BF16)

        in_maps.append(
            {
                "xTa": xTa,
                "wqk": wqk,
                "wv": wv,
                "wpr": np.ascontiguousarray(wpr),
                "cosT2": cosT2,
                "sinT2": sinT2,
                "mask33": mask33,
                "ones4": ones4,
                "perm": permm,
            }
        )
    return in_maps


_NC_CACHE = None


def kernel(x, Wqkv, bqkv, qg, kg, Wproj, bproj, cos, sin):
    global LAST_RESULTS, _NC_CACHE
    from concourse.bass_utils import run_bass_kernel_spmd

    if _NC_CACHE is None:
        _NC_CACHE = _build_bass()
    nc = _NC_CACHE

    in_maps = _host_inputs(x, Wqkv, bqkv, qg, kg, Wproj, cos, sin)
    res = run_bass_kernel_spmd(nc, in_maps, core_ids=list(range(N_CORES)))
    LAST_RESULTS = res

    bproj = np.asarray(bproj, dtype=np.float32)
    out = np.zeros((B, S, D), dtype=np.float32)
    for b in range(B):
        acc = np.zeros((S, D), dtype=np.float32)
        for i in range(4):
            acc += res.results[4 * b + i]["out"]
        out[b] = acc + bproj[None, :]
    return out
